# revision 15
# baseline (speedup 1.0000x reference)
"""BYOL-style cosine MSE loss on 8 Trainium2 NeuronCores.

Full inputs: online_output [16384, 1024] f32, target_output [16384, 1024] f32.
Output: scalar f32 = mean(2 - 2*cos_row(online, target)) / 0.05.

Sharding: data-parallel along N. Each of the 8 cores gets 2048 rows. Rows are
mapped row = p*16 + t (partition-major), so a 2-tile DMA chunk is 8 KiB
contiguous per partition (half the descriptors of the tile-major layout).
Per row r the kernel computes dot_r = sum_d o*t, n1sq_r = sum_d o*o,
n2sq_r = sum_d t*t via fused multiply-reduce ops, overlapped with HWDGE DMA
loads under a Tile pipeline. Job split per 2-tile chunk: DVE gets the 2 dots
+ 1 square, ACT gets 3 squares (DVE ~1.38us/job vs ACT ~1.49us/job keeps both
streams ~34us, under the ~41us DMA stream @ ~410 GB/s per-core cap). Per-row
stats return to the host, which finishes cosine + mean in float64 (the
"all-reduce" is a trivial 8-way host reduction of ~25 KB/core).

The NEFF is compiled with --max-sem-num so walrus's preamble/postamble
semaphore-zeroing loops cover only the ~16 compacted sems actually used
instead of all 256 (saves several us on both ends).
"""

import numpy as np

P = 128          # SBUF partitions
D = 1024         # feature dim
N = 16384        # total rows
N_CORES = 8
N_LOC = N // N_CORES          # 2048 rows per core
N_TILES = N_LOC // P          # 16 row-tiles per core
HALF = N_TILES // 2

TEMP = 0.05
EPS = 1e-8

_NC_CACHE = {}
_MAX_SEM = {"n": 40}
_DROP_DRAIN = True


# --- compile-flag plumbing -------------------------------------------------
# walrus zeroes every semaphore below --max-sem-num in its NEFF preamble and
# postamble (one EventSemaphore per sem, split across engines, ~tens of ns
# each). Default covers all 256 sems (~3 us preamble + ~7 us postamble); our
# kernel's sems are compacted to ids 3..~17, so cap the loop.
def _install_walrus_flag_patch():
    import concourse.bass_utils as bu

    if getattr(bu.run_command, "_byol_patched", False):
        return
    orig = bu.run_command

    def patched(argv, **kwargs):
        if (
            isinstance(argv, list)
            and argv
            and "walrus_driver" in str(argv[0])
            and "--neff-output-filename" in argv
        ):
            argv = list(argv) + [f"--max-sem-num={_MAX_SEM['n']}"]
        return orig(argv, **kwargs)

    patched._byol_patched = True
    bu.run_command = patched


def _legalize_waits(nc, max_waits=1):
    """Split multi-wait instructions into single-wait NOPs + the instruction.

    This container's walrus build accepts at most one semaphore wait per
    instruction, while Tile emits instructions waiting on several producer
    sems. AND-of-waits is preserved by stalling the same engine's sequencer
    on a chain of single-wait NOPs immediately before the instruction.
    """
    import concourse.mybir as mybir

    ctr = 0
    for f in nc.m.functions:
        for b in f.blocks:
            ins_list = b.instructions
            i = 0
            while i < len(ins_list):
                inst = ins_list[i]
                si = inst.sync_info
                if (
                    si is not None
                    and si.on_wait is not None
                    and len(si.on_wait) > max_waits
                ):
                    waits = si.on_wait
                    extra = [waits.pop() for _ in range(len(waits) - max_waits)]
                    for w in reversed(extra):
                        ctr += 1
                        noop = mybir.InstNoOp(
                            name=f"waitsplit_{ctr}",
                            engine=inst.engine,
                            ins=[],
                            outs=[],
                            sync_info=mybir.SyncInfo(on_wait=[w], on_update=[]),
                        )
                        ins_list.insert(i, noop)
                        i += 1
                i += 1


def _trim_tail_barrier(nc):
    """Shrink the TileContext exit sequence to just the SP DMA-drain.

    Tile emits: drain -> all-engine barrier -> sem clears (Pool ISA) ->
    all-engine barrier. Everything after the drain exists to leave the
    semaphores cleared for the NEXT execution; instead, relocate the clear
    to the kernel START (on Pool, before the existing start barrier, so
    every engine's first sem use still happens after the clear) and delete
    both exit barriers. The SP drain must stay: it waits for the DMA queues,
    guaranteeing the stats write landed before the NEFF completes.
    """
    import concourse.mybir as mybir

    moved = []
    for f in nc.m.functions:
        end_blocks = [b for b in f.blocks if b.name.endswith("_end")]
        main_blocks = [b for b in f.blocks if b.name == "main"]
        if not end_blocks or not main_blocks:
            continue
        ins_list = end_blocks[0].instructions
        moved = [
            ins
            for ins in ins_list
            if isinstance(ins, mybir.InstISA)
            and ins.engine == mybir.EngineType.Pool
        ]
        for i, ins in enumerate(ins_list):
            if isinstance(ins, mybir.InstDrain) and ins.engine == mybir.EngineType.SP:
                del ins_list[i + 1 :]
                if _DROP_DRAIN:
                    # Drop the receipt wait too: the stats write lands ~1.5 us
                    # after issue, while the walrus postamble (barriers + sem
                    # clears, >=6 us) must still run before the NEFF can
                    # complete — the data is down long before the host can
                    # see "done".
                    del ins_list[i]
                break
        main_ins = main_blocks[0].instructions
        for i, ins in enumerate(main_ins):
            if (
                isinstance(ins, mybir.InstEventSemaphore)
                and ins.engine == mybir.EngineType.Pool
            ):
                for k, m in enumerate(moved):
                    main_ins.insert(i + k, m)
                break
    return nc


def _hoist_first_loads(nc, n_hoist=2):
    """Issue the first chunk's loads before SP joins the start barrier.

    The start barrier gates every engine (via Pool's release) on all engine
    preambles finishing, putting SP's first DMA issue ~1 us later than
    necessary. SP's first loads have no waits (fresh buffers) and no
    dependency on the other engines' preambles, so hoist them into `main`
    just before SP's barrier drain. Their completion sems increment ~3 us
    after Pool's const memsets / sem clears retire, so there is no
    clear/increment race. SP still participates in the barrier, keeping
    walrus's thresholds valid.
    """
    import concourse.mybir as mybir

    for f in nc.m.functions:
        main_blocks = [b for b in f.blocks if b.name == "main"]
        body_blocks = [
            b for b in f.blocks if b.name != "main" and not b.name.endswith("_end")
        ]
        if not main_blocks or not body_blocks:
            continue
        body = body_blocks[0].instructions
        hoisted = []
        i = 0
        while i < len(body) and len(hoisted) < n_hoist:
            ins = body[i]
            if (
                isinstance(ins, mybir.InstDMACopy)
                and ins.engine == mybir.EngineType.SP
            ):
                si = ins.sync_info
                if si is not None and si.on_wait:
                    break  # only waitless leading loads are safe to hoist
                hoisted.append(ins)
                del body[i]
                continue
            i += 1
        if not hoisted:
            continue
        main_ins = main_blocks[0].instructions
        for i, ins in enumerate(main_ins):
            if ins.engine == mybir.EngineType.SP and isinstance(
                ins, (mybir.InstDrain, mybir.InstEventSemaphore)
            ):
                for k, m in enumerate(hoisted):
                    main_ins.insert(i + k, m)
                break
    return nc


def _compact_sems(nc, keep_below=3, base=3):
    """Densely remap semaphore ids to start at `base`.

    Bass allocates sem ids from a pool starting around 150; compacting to
    3..~17 lets --max-sem-num cap walrus's sem-zeroing loops.
    """
    mapping = {}
    for f in nc.m.functions:
        for b in f.blocks:
            for ins in b.instructions:
                si = ins.sync_info
                if not si:
                    continue
                for lst in (si.on_wait, si.on_update):
                    if not lst:
                        continue
                    for e in lst:
                        i = getattr(e, "id", None)
                        if i is None or i < keep_below:
                            continue
                        if i not in mapping:
                            mapping[i] = base + len(mapping)
                        e.id = mapping[i]
    return (max(mapping.values()) + 1) if mapping else base


def _slim_exit_drain(nc):
    """Keep only the stats-DMA completion waits on the exit drain.

    Tile's exit drain waits on every sem lane the kernel touched. All of
    them except the output DMAs' completion lanes are transitively implied:
    the stats DMAs' own waits required all compute, which required all
    input loads.
    """
    import concourse.mybir as mybir

    for f in nc.m.functions:
        end_blocks = [b for b in f.blocks if b.name.endswith("_end")]
        body_blocks = [
            b for b in f.blocks if not b.name.endswith("_end") and b.name != "main"
        ]
        if not end_blocks:
            continue
        stats_lanes = set()
        for b in body_blocks:
            for ins in b.instructions:
                if not isinstance(ins, mybir.InstDMACopy):
                    continue
                outs = getattr(ins, "outs", [])
                if not any("stats" in str(getattr(o, "memref", "")) for o in outs):
                    continue
                si = ins.sync_info
                if si and si.on_update:
                    for u in si.on_update:
                        stats_lanes.add(u.id)
        if not stats_lanes:
            continue
        for b in end_blocks:
            for ins in b.instructions:
                if (
                    isinstance(ins, mybir.InstDrain)
                    and ins.engine == mybir.EngineType.SP
                ):
                    si = ins.sync_info
                    if si and si.on_wait:
                        kept = [w for w in si.on_wait if w.id in stats_lanes]
                        if kept:
                            while len(si.on_wait) > 0:
                                si.on_wait.pop()
                            for w in kept:
                                si.on_wait.append(w)
                    break
    return nc


def _build_nc():
    import concourse.bass as bass
    import concourse.mybir as mybir
    from concourse.tile import TileContext

    fp32 = mybir.dt.float32
    Sq = mybir.ActivationFunctionType.Square
    mult = mybir.AluOpType.mult

    # chunk schedule: small first chunk so compute starts early, 4-tile
    # (16 KiB/partition-line) loads through the bulk for DMA-engine packet
    # efficiency, tile 14 single, tile 15 as two half-D loads so the
    # post-DMA compute tail is as short as possible
    chunks = [2, 4, 4, 4, 1]
    assert sum(chunks) == N_TILES - 1

    nc = bass.Bass(enable_partition_id=False)
    o_in = nc.declare_dram_parameter("online", [N_LOC, D], fp32, isOutput=False)
    t_in = nc.declare_dram_parameter("target", [N_LOC, D], fp32, isOutput=False)
    # stats0[:, 3*tt+k] = (dot, sum o^2, sum t^2)[k] of tile tt (0..7);
    # stats1[:, 3*tt+k] likewise for tiles 8+tt (tt 0..6); stats2[:, 3*j+k]
    # holds tile 15's per-half-D partial sums (j=0: d<512, j=1: d>=512) —
    # the host adds the two halves.
    stats0 = nc.declare_dram_parameter("stats0", [P, 3 * HALF], fp32, isOutput=True)
    stats1 = nc.declare_dram_parameter("stats1", [P, 3 * (HALF - 1)], fp32, isOutput=True)
    stats2 = nc.declare_dram_parameter("stats2", [P, 6], fp32, isOutput=True)

    # row = p*N_TILES + t: per-partition lines of a multi-tile chunk are
    # contiguous in HBM
    o_all = o_in.rearrange("(p t) d -> p t d", p=P)
    t_all = t_in.rearrange("(p t) d -> p t d", p=P)

    with TileContext(nc) as tc:
        with (
            tc.tile_pool(name="io", bufs=3) as io_pool,
            tc.tile_pool(name="scr", bufs=2) as scr_pool,
            tc.tile_pool(name="acc", bufs=1) as acc_pool,
        ):
            accs = [
                acc_pool.tile([P, 3 * HALF], fp32, name="acc0", tag="acc0"),
                acc_pool.tile([P, 3 * (HALF - 1)], fp32, name="acc1", tag="acc1"),
                acc_pool.tile([P, 6], fp32, name="acc2", tag="acc2"),
            ]
            t0 = 0
            for cg in chunks:
                o_tile = io_pool.tile([P, 4 * D], fp32, name="o_tile")
                t_tile = io_pool.tile([P, 4 * D], fp32, name="t_tile")
                nc.sync.dma_start(
                    out=o_tile[:, 0 : cg * D].rearrange("p (t d) -> p t d", t=cg),
                    in_=o_all[:, t0 : t0 + cg],
                )
                nc.sync.dma_start(
                    out=t_tile[:, 0 : cg * D].rearrange("p (t d) -> p t d", t=cg),
                    in_=t_all[:, t0 : t0 + cg],
                )
                for gi in range(cg):
                    idx = t0 + gi
                    h = idx // HALF
                    col = 3 * (idx % HALF)
                    acc = accs[h]
                    osl = o_tile[:, gi * D : (gi + 1) * D]
                    tsl = t_tile[:, gi * D : (gi + 1) * D]
                    prod = scr_pool.tile([P, D], fp32, name="prod")
                    sq_o = scr_pool.tile([P, D], fp32, name="sq_o")
                    sq_t = scr_pool.tile([P, D], fp32, name="sq_t")
                    # dot: always DVE
                    nc.vector.scalar_tensor_tensor(
                        out=prod[:],
                        in0=osl,
                        scalar=1.0,
                        in1=tsl,
                        op0=mult,
                        op1=mult,
                        accum_out=acc[:, col : col + 1],
                    )
                    # o^2: always ACT
                    nc.scalar.activation(
                        sq_o[:], osl, Sq, accum_out=acc[:, col + 1 : col + 2]
                    )
                    # t^2: odd tiles on DVE, even on ACT (3 ACT / 3 DVE jobs
                    # per 2-tile chunk would overload DVE with dots; this
                    # gives DVE 15 dots + 7 squares, ACT 22 squares over
                    # tiles 0..14).
                    if idx % 2 == 1:
                        nc.vector.scalar_tensor_tensor(
                            out=sq_t[:],
                            in0=tsl,
                            scalar=1.0,
                            in1=tsl,
                            op0=mult,
                            op1=mult,
                            accum_out=acc[:, col + 2 : col + 3],
                        )
                    else:
                        nc.scalar.activation(
                            sq_t[:], tsl, Sq, accum_out=acc[:, col + 2 : col + 3]
                        )
                t0 += cg
            # tile 15 as two half-D loads; per half: dot on DVE, o^2 on ACT,
            # t^2 on DVE (h0) / ACT (h1) — both engines finish ~1.4 us after
            # the last bytes land
            hw = D // 2
            last = N_TILES - 1
            o_tile = io_pool.tile([P, 4 * D], fp32, name="o_tile")
            t_tile = io_pool.tile([P, 4 * D], fp32, name="t_tile")
            acc = accs[2]
            for j in range(2):
                dsl = slice(j * hw, (j + 1) * hw)
                nc.sync.dma_start(out=o_tile[:, j * hw : (j + 1) * hw], in_=o_all[:, last, dsl])
                nc.sync.dma_start(out=t_tile[:, j * hw : (j + 1) * hw], in_=t_all[:, last, dsl])
            for j in range(2):
                osl = o_tile[:, j * hw : (j + 1) * hw]
                tsl = t_tile[:, j * hw : (j + 1) * hw]
                prod = scr_pool.tile([P, hw], fp32, name="prod")
                sq_o = scr_pool.tile([P, hw], fp32, name="sq_o")
                sq_t = scr_pool.tile([P, hw], fp32, name="sq_t")
                nc.vector.scalar_tensor_tensor(
                    out=prod[:],
                    in0=osl,
                    scalar=1.0,
                    in1=tsl,
                    op0=mult,
                    op1=mult,
                    accum_out=acc[:, 3 * j : 3 * j + 1],
                )
                nc.scalar.activation(
                    sq_o[:], osl, Sq, accum_out=acc[:, 3 * j + 1 : 3 * j + 2]
                )
                if j == 0:
                    nc.vector.scalar_tensor_tensor(
                        out=sq_t[:],
                        in0=tsl,
                        scalar=1.0,
                        in1=tsl,
                        op0=mult,
                        op1=mult,
                        accum_out=acc[:, 3 * j + 2 : 3 * j + 3],
                    )
                else:
                    nc.scalar.activation(
                        sq_t[:], tsl, Sq, accum_out=acc[:, 3 * j + 2 : 3 * j + 3]
                    )
            # Emit the stats DMAs after every load so their compute-waits
            # stall the SP sequencer only once it has nothing left to issue.
            # stats0/1 still execute as soon as their tiles finish; stats2
            # (2 KiB) is the only write on the critical tail.
            nc.sync.dma_start(out=stats0[:, :], in_=accs[0][:])
            nc.sync.dma_start(out=stats1[:, :], in_=accs[1][:])
            nc.sync.dma_start(out=stats2[:, :], in_=accs[2][:])

    _trim_tail_barrier(nc)
    _hoist_first_loads(nc)
    _slim_exit_drain(nc)
    _legalize_waits(nc)
    _MAX_SEM["n"] = _compact_sems(nc) + 8  # headroom for walrus-internal sems
    return nc


def _get_nc():
    if "nc" not in _NC_CACHE:
        _install_walrus_flag_patch()
        _NC_CACHE["nc"] = _build_nc()
    return _NC_CACHE["nc"]


def _run_device(online_output, target_output, **spmd_kwargs):
    """Shard inputs, run the SPMD kernel, return per-core stats + raw result."""
    from concourse.bass_utils import run_bass_kernel_spmd

    nc = _get_nc()
    in_maps = []
    for c in range(N_CORES):
        sl = slice(c * N_LOC, (c + 1) * N_LOC)
        in_maps.append(
            {
                "online": np.ascontiguousarray(online_output[sl], dtype=np.float32),
                "target": np.ascontiguousarray(target_output[sl], dtype=np.float32),
            }
        )
    res = run_bass_kernel_spmd(nc, in_maps, list(range(N_CORES)), **spmd_kwargs)
    return res


def _finish_host(results):
    """Gather per-core stats and finish the cosine + mean in float64."""
    dots, n1s, n2s = [], [], []
    for i in range(N_CORES):
        st0 = np.asarray(results[i]["stats0"], dtype=np.float64)  # [P, 24]
        st1 = np.asarray(results[i]["stats1"], dtype=np.float64)  # [P, 21]
        st2 = np.asarray(results[i]["stats2"], dtype=np.float64)  # [P, 6]
        a0 = st0.reshape(P, HALF, 3)
        a1 = st1.reshape(P, HALF - 1, 3)
        a2 = (st2[:, 0:3] + st2[:, 3:6]).reshape(P, 1, 3)  # tile 15 halves
        # row_local = p*16 + t  ->  [P, 16, 3] flattens to row-major
        a = np.concatenate([a0, a1, a2], axis=1).reshape(-1, 3)
        dots.append(a[:, 0])
        n1s.append(a[:, 1])
        n2s.append(a[:, 2])
    dot = np.concatenate(dots)
    n1 = np.sqrt(np.concatenate(n1s))
    n2 = np.sqrt(np.concatenate(n2s))
    cos = dot / (np.maximum(n1, EPS) * np.maximum(n2, EPS))
    return np.array((2.0 - 2.0 * cos).mean() / TEMP, dtype=np.float32)


def kernel(online_output, target_output):
    res = _run_device(online_output, target_output)
    return _finish_host(res.results)


# revision 16
# speedup vs baseline: 1.0025x; 1.0025x over previous
"""BYOL-style cosine MSE loss on 8 Trainium2 NeuronCores.

Full inputs: online_output [16384, 1024] f32, target_output [16384, 1024] f32.
Output: scalar f32 = mean(2 - 2*cos_row(online, target)) / 0.05.

Sharding: data-parallel along N. Each of the 8 cores gets 2048 rows. Rows are
mapped row = p*16 + t (partition-major), so a 2-tile DMA chunk is 8 KiB
contiguous per partition (half the descriptors of the tile-major layout).
Per row r the kernel computes dot_r = sum_d o*t, n1sq_r = sum_d o*o,
n2sq_r = sum_d t*t via fused multiply-reduce ops, overlapped with HWDGE DMA
loads under a Tile pipeline. Job split per 2-tile chunk: DVE gets the 2 dots
+ 1 square, ACT gets 3 squares (DVE ~1.38us/job vs ACT ~1.49us/job keeps both
streams ~34us, under the ~41us DMA stream @ ~410 GB/s per-core cap). Per-row
stats return to the host, which finishes cosine + mean in float64 (the
"all-reduce" is a trivial 8-way host reduction of ~25 KB/core).

The NEFF is compiled with --max-sem-num so walrus's preamble/postamble
semaphore-zeroing loops cover only the ~16 compacted sems actually used
instead of all 256 (saves several us on both ends).
"""

import numpy as np

P = 128          # SBUF partitions
D = 1024         # feature dim
N = 16384        # total rows
N_CORES = 8
N_LOC = N // N_CORES          # 2048 rows per core
N_TILES = N_LOC // P          # 16 row-tiles per core
HALF = N_TILES // 2

TEMP = 0.05
EPS = 1e-8

_NC_CACHE = {}
_MAX_SEM = {"n": 40}
_DROP_DRAIN = True


# --- compile-flag plumbing -------------------------------------------------
# walrus zeroes every semaphore below --max-sem-num in its NEFF preamble and
# postamble (one EventSemaphore per sem, split across engines, ~tens of ns
# each). Default covers all 256 sems (~3 us preamble + ~7 us postamble); our
# kernel's sems are compacted to ids 3..~17, so cap the loop.
def _install_walrus_flag_patch():
    import concourse.bass_utils as bu

    if getattr(bu.run_command, "_byol_patched", False):
        return
    orig = bu.run_command

    def patched(argv, **kwargs):
        if (
            isinstance(argv, list)
            and argv
            and "walrus_driver" in str(argv[0])
            and "--neff-output-filename" in argv
        ):
            argv = list(argv) + [f"--max-sem-num={_MAX_SEM['n']}"]
        return orig(argv, **kwargs)

    patched._byol_patched = True
    bu.run_command = patched


def _legalize_waits(nc, max_waits=1):
    """Split multi-wait instructions into single-wait NOPs + the instruction.

    This container's walrus build accepts at most one semaphore wait per
    instruction, while Tile emits instructions waiting on several producer
    sems. AND-of-waits is preserved by stalling the same engine's sequencer
    on a chain of single-wait NOPs immediately before the instruction.
    """
    import concourse.mybir as mybir

    ctr = 0
    for f in nc.m.functions:
        for b in f.blocks:
            ins_list = b.instructions
            i = 0
            while i < len(ins_list):
                inst = ins_list[i]
                si = inst.sync_info
                if (
                    si is not None
                    and si.on_wait is not None
                    and len(si.on_wait) > max_waits
                ):
                    waits = si.on_wait
                    extra = [waits.pop() for _ in range(len(waits) - max_waits)]
                    for w in reversed(extra):
                        ctr += 1
                        noop = mybir.InstNoOp(
                            name=f"waitsplit_{ctr}",
                            engine=inst.engine,
                            ins=[],
                            outs=[],
                            sync_info=mybir.SyncInfo(on_wait=[w], on_update=[]),
                        )
                        ins_list.insert(i, noop)
                        i += 1
                i += 1


def _trim_tail_barrier(nc):
    """Shrink the TileContext exit sequence to just the SP DMA-drain.

    Tile emits: drain -> all-engine barrier -> sem clears (Pool ISA) ->
    all-engine barrier. Everything after the drain exists to leave the
    semaphores cleared for the NEXT execution; instead, relocate the clear
    to the kernel START (on Pool, before the existing start barrier, so
    every engine's first sem use still happens after the clear) and delete
    both exit barriers. The SP drain must stay: it waits for the DMA queues,
    guaranteeing the stats write landed before the NEFF completes.
    """
    import concourse.mybir as mybir

    moved = []
    for f in nc.m.functions:
        end_blocks = [b for b in f.blocks if b.name.endswith("_end")]
        main_blocks = [b for b in f.blocks if b.name == "main"]
        if not end_blocks or not main_blocks:
            continue
        ins_list = end_blocks[0].instructions
        moved = [
            ins
            for ins in ins_list
            if isinstance(ins, mybir.InstISA)
            and ins.engine == mybir.EngineType.Pool
        ]
        for i, ins in enumerate(ins_list):
            if isinstance(ins, mybir.InstDrain) and ins.engine == mybir.EngineType.SP:
                del ins_list[i + 1 :]
                if _DROP_DRAIN:
                    # Drop the receipt wait too: the stats write lands ~1.5 us
                    # after issue, while the walrus postamble (barriers + sem
                    # clears, >=6 us) must still run before the NEFF can
                    # complete — the data is down long before the host can
                    # see "done".
                    del ins_list[i]
                break
        main_ins = main_blocks[0].instructions
        for i, ins in enumerate(main_ins):
            if (
                isinstance(ins, mybir.InstEventSemaphore)
                and ins.engine == mybir.EngineType.Pool
            ):
                for k, m in enumerate(moved):
                    main_ins.insert(i + k, m)
                break
    return nc


def _hoist_first_loads(nc, n_hoist=2):
    """Issue the first chunk's loads before SP joins the start barrier.

    The start barrier gates every engine (via Pool's release) on all engine
    preambles finishing, putting SP's first DMA issue ~1 us later than
    necessary. SP's first loads have no waits (fresh buffers) and no
    dependency on the other engines' preambles, so hoist them into `main`
    just before SP's barrier drain. Their completion sems increment ~3 us
    after Pool's const memsets / sem clears retire, so there is no
    clear/increment race. SP still participates in the barrier, keeping
    walrus's thresholds valid.
    """
    import concourse.mybir as mybir

    for f in nc.m.functions:
        main_blocks = [b for b in f.blocks if b.name == "main"]
        body_blocks = [
            b for b in f.blocks if b.name != "main" and not b.name.endswith("_end")
        ]
        if not main_blocks or not body_blocks:
            continue
        body = body_blocks[0].instructions
        hoisted = []
        i = 0
        while i < len(body) and len(hoisted) < n_hoist:
            ins = body[i]
            if (
                isinstance(ins, mybir.InstDMACopy)
                and ins.engine == mybir.EngineType.SP
            ):
                si = ins.sync_info
                if si is not None and si.on_wait:
                    break  # only waitless leading loads are safe to hoist
                hoisted.append(ins)
                del body[i]
                continue
            i += 1
        if not hoisted:
            continue
        main_ins = main_blocks[0].instructions
        for i, ins in enumerate(main_ins):
            if ins.engine == mybir.EngineType.SP and isinstance(
                ins, (mybir.InstDrain, mybir.InstEventSemaphore)
            ):
                for k, m in enumerate(hoisted):
                    main_ins.insert(i + k, m)
                break
    return nc


def _compact_sems(nc, keep_below=3, base=3):
    """Densely remap semaphore ids to start at `base`.

    Bass allocates sem ids from a pool starting around 150; compacting to
    3..~17 lets --max-sem-num cap walrus's sem-zeroing loops.
    """
    mapping = {}
    for f in nc.m.functions:
        for b in f.blocks:
            for ins in b.instructions:
                si = ins.sync_info
                if not si:
                    continue
                for lst in (si.on_wait, si.on_update):
                    if not lst:
                        continue
                    for e in lst:
                        i = getattr(e, "id", None)
                        if i is None or i < keep_below:
                            continue
                        if i not in mapping:
                            mapping[i] = base + len(mapping)
                        e.id = mapping[i]
    return (max(mapping.values()) + 1) if mapping else base


def _slim_exit_drain(nc):
    """Keep only the stats-DMA completion waits on the exit drain.

    Tile's exit drain waits on every sem lane the kernel touched. All of
    them except the output DMAs' completion lanes are transitively implied:
    the stats DMAs' own waits required all compute, which required all
    input loads.
    """
    import concourse.mybir as mybir

    for f in nc.m.functions:
        end_blocks = [b for b in f.blocks if b.name.endswith("_end")]
        body_blocks = [
            b for b in f.blocks if not b.name.endswith("_end") and b.name != "main"
        ]
        if not end_blocks:
            continue
        stats_lanes = set()
        for b in body_blocks:
            for ins in b.instructions:
                if not isinstance(ins, mybir.InstDMACopy):
                    continue
                outs = getattr(ins, "outs", [])
                if not any("stats" in str(getattr(o, "memref", "")) for o in outs):
                    continue
                si = ins.sync_info
                if si and si.on_update:
                    for u in si.on_update:
                        stats_lanes.add(u.id)
        if not stats_lanes:
            continue
        for b in end_blocks:
            for ins in b.instructions:
                if (
                    isinstance(ins, mybir.InstDrain)
                    and ins.engine == mybir.EngineType.SP
                ):
                    si = ins.sync_info
                    if si and si.on_wait:
                        kept = [w for w in si.on_wait if w.id in stats_lanes]
                        if kept:
                            while len(si.on_wait) > 0:
                                si.on_wait.pop()
                            for w in kept:
                                si.on_wait.append(w)
                    break
    return nc


def _build_nc():
    import concourse.bass as bass
    import concourse.mybir as mybir
    from concourse.tile import TileContext

    fp32 = mybir.dt.float32
    Sq = mybir.ActivationFunctionType.Square
    mult = mybir.AluOpType.mult

    # chunk schedule: small first chunk so compute starts early, 4-tile
    # (16 KiB/partition-line) loads through the bulk for DMA-engine packet
    # efficiency, tile 14 single, tile 15 as two half-D loads so the
    # post-DMA compute tail is as short as possible
    chunks = [2, 4, 4, 4, 1]
    assert sum(chunks) == N_TILES - 1

    nc = bass.Bass(enable_partition_id=False)
    o_in = nc.declare_dram_parameter("online", [N_LOC, D], fp32, isOutput=False)
    t_in = nc.declare_dram_parameter("target", [N_LOC, D], fp32, isOutput=False)
    # stats0[:, 3*tt+k] = (dot, sum o^2, sum t^2)[k] of tile tt (0..7);
    # stats1[:, 3*tt+k] likewise for tiles 8+tt (tt 0..6); stats2[:, 3*j+k]
    # holds tile 15's per-half-D partial sums (j=0: d<512, j=1: d>=512) —
    # the host adds the two halves.
    stats0 = nc.declare_dram_parameter("stats0", [P, 3 * HALF], fp32, isOutput=True)
    stats1 = nc.declare_dram_parameter("stats1", [P, 3 * (HALF - 1)], fp32, isOutput=True)
    stats2 = nc.declare_dram_parameter("stats2", [P, 6], fp32, isOutput=True)

    # row = p*N_TILES + t: per-partition lines of a multi-tile chunk are
    # contiguous in HBM
    o_all = o_in.rearrange("(p t) d -> p t d", p=P)
    t_all = t_in.rearrange("(p t) d -> p t d", p=P)

    with TileContext(nc) as tc:
        with (
            tc.tile_pool(name="io", bufs=4) as io_pool,
            tc.tile_pool(name="scr", bufs=2) as scr_pool,
            tc.tile_pool(name="acc", bufs=1) as acc_pool,
        ):
            accs = [
                acc_pool.tile([P, 3 * HALF], fp32, name="acc0", tag="acc0"),
                acc_pool.tile([P, 3 * (HALF - 1)], fp32, name="acc1", tag="acc1"),
                acc_pool.tile([P, 6], fp32, name="acc2", tag="acc2"),
            ]
            t0 = 0
            for cg in chunks:
                o_tile = io_pool.tile([P, 4 * D], fp32, name="o_tile")
                t_tile = io_pool.tile([P, 4 * D], fp32, name="t_tile")
                nc.sync.dma_start(
                    out=o_tile[:, 0 : cg * D].rearrange("p (t d) -> p t d", t=cg),
                    in_=o_all[:, t0 : t0 + cg],
                )
                nc.sync.dma_start(
                    out=t_tile[:, 0 : cg * D].rearrange("p (t d) -> p t d", t=cg),
                    in_=t_all[:, t0 : t0 + cg],
                )
                for gi in range(cg):
                    idx = t0 + gi
                    h = idx // HALF
                    col = 3 * (idx % HALF)
                    acc = accs[h]
                    osl = o_tile[:, gi * D : (gi + 1) * D]
                    tsl = t_tile[:, gi * D : (gi + 1) * D]
                    prod = scr_pool.tile([P, D], fp32, name="prod")
                    sq_o = scr_pool.tile([P, D], fp32, name="sq_o")
                    sq_t = scr_pool.tile([P, D], fp32, name="sq_t")
                    # dot: always DVE
                    nc.vector.scalar_tensor_tensor(
                        out=prod[:],
                        in0=osl,
                        scalar=1.0,
                        in1=tsl,
                        op0=mult,
                        op1=mult,
                        accum_out=acc[:, col : col + 1],
                    )
                    # o^2: always ACT
                    nc.scalar.activation(
                        sq_o[:], osl, Sq, accum_out=acc[:, col + 1 : col + 2]
                    )
                    # t^2: odd tiles on DVE, even on ACT (3 ACT / 3 DVE jobs
                    # per 2-tile chunk would overload DVE with dots; this
                    # gives DVE 15 dots + 7 squares, ACT 22 squares over
                    # tiles 0..14).
                    if idx % 2 == 1 or idx in (6, 12):
                        nc.vector.scalar_tensor_tensor(
                            out=sq_t[:],
                            in0=tsl,
                            scalar=1.0,
                            in1=tsl,
                            op0=mult,
                            op1=mult,
                            accum_out=acc[:, col + 2 : col + 3],
                        )
                    else:
                        nc.scalar.activation(
                            sq_t[:], tsl, Sq, accum_out=acc[:, col + 2 : col + 3]
                        )
                t0 += cg
            # tile 15 as two half-D loads; per half: dot on DVE, o^2 on ACT,
            # t^2 on DVE (h0) / ACT (h1) — both engines finish ~1.4 us after
            # the last bytes land
            hw = D // 2
            last = N_TILES - 1
            o_tile = io_pool.tile([P, 4 * D], fp32, name="o_tile")
            t_tile = io_pool.tile([P, 4 * D], fp32, name="t_tile")
            acc = accs[2]
            for j in range(2):
                dsl = slice(j * hw, (j + 1) * hw)
                nc.sync.dma_start(out=o_tile[:, j * hw : (j + 1) * hw], in_=o_all[:, last, dsl])
                nc.sync.dma_start(out=t_tile[:, j * hw : (j + 1) * hw], in_=t_all[:, last, dsl])
            for j in range(2):
                osl = o_tile[:, j * hw : (j + 1) * hw]
                tsl = t_tile[:, j * hw : (j + 1) * hw]
                prod = scr_pool.tile([P, hw], fp32, name="prod")
                sq_o = scr_pool.tile([P, hw], fp32, name="sq_o")
                sq_t = scr_pool.tile([P, hw], fp32, name="sq_t")
                nc.vector.scalar_tensor_tensor(
                    out=prod[:],
                    in0=osl,
                    scalar=1.0,
                    in1=tsl,
                    op0=mult,
                    op1=mult,
                    accum_out=acc[:, 3 * j : 3 * j + 1],
                )
                nc.scalar.activation(
                    sq_o[:], osl, Sq, accum_out=acc[:, 3 * j + 1 : 3 * j + 2]
                )
                if j == 0:
                    nc.vector.scalar_tensor_tensor(
                        out=sq_t[:],
                        in0=tsl,
                        scalar=1.0,
                        in1=tsl,
                        op0=mult,
                        op1=mult,
                        accum_out=acc[:, 3 * j + 2 : 3 * j + 3],
                    )
                else:
                    nc.scalar.activation(
                        sq_t[:], tsl, Sq, accum_out=acc[:, 3 * j + 2 : 3 * j + 3]
                    )
            # Emit the stats DMAs after every load so their compute-waits
            # stall the SP sequencer only once it has nothing left to issue.
            # stats0/1 still execute as soon as their tiles finish; stats2
            # (2 KiB) is the only write on the critical tail.
            nc.sync.dma_start(out=stats0[:, :], in_=accs[0][:])
            nc.sync.dma_start(out=stats1[:, :], in_=accs[1][:])
            nc.sync.dma_start(out=stats2[:, :], in_=accs[2][:])

    _trim_tail_barrier(nc)
    _hoist_first_loads(nc)
    _slim_exit_drain(nc)
    _legalize_waits(nc)
    _MAX_SEM["n"] = _compact_sems(nc) + 8  # headroom for walrus-internal sems
    return nc


def _get_nc():
    if "nc" not in _NC_CACHE:
        _install_walrus_flag_patch()
        _NC_CACHE["nc"] = _build_nc()
    return _NC_CACHE["nc"]


def _run_device(online_output, target_output, **spmd_kwargs):
    """Shard inputs, run the SPMD kernel, return per-core stats + raw result."""
    from concourse.bass_utils import run_bass_kernel_spmd

    nc = _get_nc()
    in_maps = []
    for c in range(N_CORES):
        sl = slice(c * N_LOC, (c + 1) * N_LOC)
        in_maps.append(
            {
                "online": np.ascontiguousarray(online_output[sl], dtype=np.float32),
                "target": np.ascontiguousarray(target_output[sl], dtype=np.float32),
            }
        )
    res = run_bass_kernel_spmd(nc, in_maps, list(range(N_CORES)), **spmd_kwargs)
    return res


def _finish_host(results):
    """Gather per-core stats and finish the cosine + mean in float64."""
    dots, n1s, n2s = [], [], []
    for i in range(N_CORES):
        st0 = np.asarray(results[i]["stats0"], dtype=np.float64)  # [P, 24]
        st1 = np.asarray(results[i]["stats1"], dtype=np.float64)  # [P, 21]
        st2 = np.asarray(results[i]["stats2"], dtype=np.float64)  # [P, 6]
        a0 = st0.reshape(P, HALF, 3)
        a1 = st1.reshape(P, HALF - 1, 3)
        a2 = (st2[:, 0:3] + st2[:, 3:6]).reshape(P, 1, 3)  # tile 15 halves
        # row_local = p*16 + t  ->  [P, 16, 3] flattens to row-major
        a = np.concatenate([a0, a1, a2], axis=1).reshape(-1, 3)
        dots.append(a[:, 0])
        n1s.append(a[:, 1])
        n2s.append(a[:, 2])
    dot = np.concatenate(dots)
    n1 = np.sqrt(np.concatenate(n1s))
    n2 = np.sqrt(np.concatenate(n2s))
    cos = dot / (np.maximum(n1, EPS) * np.maximum(n2, EPS))
    return np.array((2.0 - 2.0 * cos).mean() / TEMP, dtype=np.float32)


def kernel(online_output, target_output):
    res = _run_device(online_output, target_output)
    return _finish_host(res.results)


# revision 17
# speedup vs baseline: 1.2092x; 1.2062x over previous
"""BYOL-style cosine MSE loss on 8 Trainium2 NeuronCores.

Full inputs: online_output [16384, 1024] f32, target_output [16384, 1024] f32.
Output: scalar f32 = mean(2 - 2*cos_row(online, target)) / 0.05.

Sharding: data-parallel along N. Each of the 8 cores gets 2048 rows. Rows are
mapped row = p*16 + t (partition-major), so a 2-tile DMA chunk is 8 KiB
contiguous per partition (half the descriptors of the tile-major layout).
Per row r the kernel computes dot_r = sum_d o*t, n1sq_r = sum_d o*o,
n2sq_r = sum_d t*t via fused multiply-reduce ops, overlapped with HWDGE DMA
loads under a Tile pipeline. Job split per 2-tile chunk: DVE gets the 2 dots
+ 1 square, ACT gets 3 squares (DVE ~1.38us/job vs ACT ~1.49us/job keeps both
streams ~34us, under the ~41us DMA stream @ ~410 GB/s per-core cap). Per-row
stats return to the host, which finishes cosine + mean in float64 (the
"all-reduce" is a trivial 8-way host reduction of ~25 KB/core).

The NEFF is compiled with --max-sem-num so walrus's preamble/postamble
semaphore-zeroing loops cover only the ~16 compacted sems actually used
instead of all 256 (saves several us on both ends).
"""

import numpy as np

P = 128          # SBUF partitions
D = 1024         # feature dim
N = 16384        # total rows
N_CORES = 8
N_LOC = N // N_CORES          # 2048 rows per core
N_TILES = N_LOC // P          # 16 row-tiles per core
HALF = N_TILES // 2

TEMP = 0.05
EPS = 1e-8

_NC_CACHE = {}
_MAX_SEM = {"n": 40}
_DROP_DRAIN = True


# --- compile-flag plumbing -------------------------------------------------
# walrus zeroes every semaphore below --max-sem-num in its NEFF preamble and
# postamble (one EventSemaphore per sem, split across engines, ~tens of ns
# each). Default covers all 256 sems (~3 us preamble + ~7 us postamble); our
# kernel's sems are compacted to ids 3..~17, so cap the loop.
def _install_walrus_flag_patch():
    import concourse.bass_utils as bu

    if getattr(bu.run_command, "_byol_patched", False):
        return
    orig = bu.run_command

    def patched(argv, **kwargs):
        if (
            isinstance(argv, list)
            and argv
            and "walrus_driver" in str(argv[0])
            and "--neff-output-filename" in argv
        ):
            argv = list(argv) + [f"--max-sem-num={_MAX_SEM['n']}"]
        return orig(argv, **kwargs)

    patched._byol_patched = True
    bu.run_command = patched


def _legalize_waits(nc, max_waits=1):
    """Split multi-wait instructions into single-wait NOPs + the instruction.

    This container's walrus build accepts at most one semaphore wait per
    instruction, while Tile emits instructions waiting on several producer
    sems. AND-of-waits is preserved by stalling the same engine's sequencer
    on a chain of single-wait NOPs immediately before the instruction.
    """
    import concourse.mybir as mybir

    ctr = 0
    for f in nc.m.functions:
        for b in f.blocks:
            ins_list = b.instructions
            i = 0
            while i < len(ins_list):
                inst = ins_list[i]
                si = inst.sync_info
                if (
                    si is not None
                    and si.on_wait is not None
                    and len(si.on_wait) > max_waits
                ):
                    waits = si.on_wait
                    extra = [waits.pop() for _ in range(len(waits) - max_waits)]
                    for w in reversed(extra):
                        ctr += 1
                        noop = mybir.InstNoOp(
                            name=f"waitsplit_{ctr}",
                            engine=inst.engine,
                            ins=[],
                            outs=[],
                            sync_info=mybir.SyncInfo(on_wait=[w], on_update=[]),
                        )
                        ins_list.insert(i, noop)
                        i += 1
                i += 1


def _trim_tail_barrier(nc):
    """Shrink the TileContext exit sequence to just the SP DMA-drain.

    Tile emits: drain -> all-engine barrier -> sem clears (Pool ISA) ->
    all-engine barrier. Everything after the drain exists to leave the
    semaphores cleared for the NEXT execution; instead, relocate the clear
    to the kernel START (on Pool, before the existing start barrier, so
    every engine's first sem use still happens after the clear) and delete
    both exit barriers. The SP drain must stay: it waits for the DMA queues,
    guaranteeing the stats write landed before the NEFF completes.
    """
    import concourse.mybir as mybir

    moved = []
    for f in nc.m.functions:
        end_blocks = [b for b in f.blocks if b.name.endswith("_end")]
        main_blocks = [b for b in f.blocks if b.name == "main"]
        if not end_blocks or not main_blocks:
            continue
        ins_list = end_blocks[0].instructions
        moved = [
            ins
            for ins in ins_list
            if isinstance(ins, mybir.InstISA)
            and ins.engine == mybir.EngineType.Pool
        ]
        for i, ins in enumerate(ins_list):
            if isinstance(ins, mybir.InstDrain) and ins.engine == mybir.EngineType.SP:
                del ins_list[i + 1 :]
                if _DROP_DRAIN:
                    # Drop the receipt wait too: the stats write lands ~1.5 us
                    # after issue, while the walrus postamble (barriers + sem
                    # clears, >=6 us) must still run before the NEFF can
                    # complete — the data is down long before the host can
                    # see "done".
                    del ins_list[i]
                break
        main_ins = main_blocks[0].instructions
        for i, ins in enumerate(main_ins):
            if (
                isinstance(ins, mybir.InstEventSemaphore)
                and ins.engine == mybir.EngineType.Pool
            ):
                for k, m in enumerate(moved):
                    main_ins.insert(i + k, m)
                break
    return nc


def _hoist_first_loads(nc, n_hoist=2):
    """Issue the first chunk's loads before SP joins the start barrier.

    The start barrier gates every engine (via Pool's release) on all engine
    preambles finishing, putting SP's first DMA issue ~1 us later than
    necessary. SP's first loads have no waits (fresh buffers) and no
    dependency on the other engines' preambles, so hoist them into `main`
    just before SP's barrier drain. Their completion sems increment ~3 us
    after Pool's const memsets / sem clears retire, so there is no
    clear/increment race. SP still participates in the barrier, keeping
    walrus's thresholds valid.
    """
    import concourse.mybir as mybir

    for f in nc.m.functions:
        main_blocks = [b for b in f.blocks if b.name == "main"]
        body_blocks = [
            b for b in f.blocks if b.name != "main" and not b.name.endswith("_end")
        ]
        if not main_blocks or not body_blocks:
            continue
        body = body_blocks[0].instructions
        hoisted = []
        i = 0
        while i < len(body) and len(hoisted) < n_hoist:
            ins = body[i]
            if (
                isinstance(ins, mybir.InstDMACopy)
                and ins.engine == mybir.EngineType.SP
            ):
                si = ins.sync_info
                if si is not None and si.on_wait:
                    break  # only waitless leading loads are safe to hoist
                hoisted.append(ins)
                del body[i]
                continue
            i += 1
        if not hoisted:
            continue
        main_ins = main_blocks[0].instructions
        for i, ins in enumerate(main_ins):
            if ins.engine == mybir.EngineType.SP and isinstance(
                ins, (mybir.InstDrain, mybir.InstEventSemaphore)
            ):
                for k, m in enumerate(hoisted):
                    main_ins.insert(i + k, m)
                break
    return nc


def _compact_sems(nc, keep_below=3, base=3):
    """Densely remap semaphore ids to start at `base`.

    Bass allocates sem ids from a pool starting around 150; compacting to
    3..~17 lets --max-sem-num cap walrus's sem-zeroing loops.
    """
    mapping = {}
    for f in nc.m.functions:
        for b in f.blocks:
            for ins in b.instructions:
                si = ins.sync_info
                if not si:
                    continue
                for lst in (si.on_wait, si.on_update):
                    if not lst:
                        continue
                    for e in lst:
                        i = getattr(e, "id", None)
                        if i is None or i < keep_below:
                            continue
                        if i not in mapping:
                            mapping[i] = base + len(mapping)
                        e.id = mapping[i]
    return (max(mapping.values()) + 1) if mapping else base


def _slim_exit_drain(nc):
    """Keep only the stats-DMA completion waits on the exit drain.

    Tile's exit drain waits on every sem lane the kernel touched. All of
    them except the output DMAs' completion lanes are transitively implied:
    the stats DMAs' own waits required all compute, which required all
    input loads.
    """
    import concourse.mybir as mybir

    for f in nc.m.functions:
        end_blocks = [b for b in f.blocks if b.name.endswith("_end")]
        body_blocks = [
            b for b in f.blocks if not b.name.endswith("_end") and b.name != "main"
        ]
        if not end_blocks:
            continue
        stats_lanes = set()
        for b in body_blocks:
            for ins in b.instructions:
                if not isinstance(ins, mybir.InstDMACopy):
                    continue
                outs = getattr(ins, "outs", [])
                if not any("stats" in str(getattr(o, "memref", "")) for o in outs):
                    continue
                si = ins.sync_info
                if si and si.on_update:
                    for u in si.on_update:
                        stats_lanes.add(u.id)
        if not stats_lanes:
            continue
        for b in end_blocks:
            for ins in b.instructions:
                if (
                    isinstance(ins, mybir.InstDrain)
                    and ins.engine == mybir.EngineType.SP
                ):
                    si = ins.sync_info
                    if si and si.on_wait:
                        kept = [w for w in si.on_wait if w.id in stats_lanes]
                        if kept:
                            while len(si.on_wait) > 0:
                                si.on_wait.pop()
                            for w in kept:
                                si.on_wait.append(w)
                    break
    return nc


def _build_nc():
    import concourse.bass as bass
    import concourse.mybir as mybir
    from concourse.tile import TileContext

    fp32 = mybir.dt.float32
    Sq = mybir.ActivationFunctionType.Square
    mult = mybir.AluOpType.mult

    # chunk schedule: 2-tile (8 KiB/partition-line) loads for tiles 0..13,
    # tile 14 single, tile 15 as two half-D loads so the post-DMA compute
    # tail is short. Bigger chunks give marginally better DMA packet
    # efficiency but coarser completion sems starve compute of work.
    chunks = [2] * ((N_TILES - 2) // 2) + [1]
    assert sum(chunks) == N_TILES - 1

    nc = bass.Bass(enable_partition_id=False)
    o_in = nc.declare_dram_parameter("online", [N_LOC, D], fp32, isOutput=False)
    t_in = nc.declare_dram_parameter("target", [N_LOC, D], fp32, isOutput=False)
    # stats0[:, 3*tt+k] = (dot, sum o^2, sum t^2)[k] of tile tt (0..7);
    # stats1[:, 3*tt+k] likewise for tiles 8+tt (tt 0..6); stats2[:, 3*j+k]
    # holds tile 15's per-half-D partial sums (j=0: d<512, j=1: d>=512) —
    # the host adds the two halves.
    stats0 = nc.declare_dram_parameter("stats0", [P, 3 * HALF], fp32, isOutput=True)
    stats1 = nc.declare_dram_parameter("stats1", [P, 3 * (HALF - 1)], fp32, isOutput=True)
    stats2 = nc.declare_dram_parameter("stats2", [P, 6], fp32, isOutput=True)

    # row = p*N_TILES + t: per-partition lines of a multi-tile chunk are
    # contiguous in HBM
    o_all = o_in.rearrange("(p t) d -> p t d", p=P)
    t_all = t_in.rearrange("(p t) d -> p t d", p=P)

    with TileContext(nc) as tc:
        with (
            tc.tile_pool(name="io", bufs=4) as io_pool,
            tc.tile_pool(name="scr", bufs=2) as scr_pool,
            tc.tile_pool(name="acc", bufs=1) as acc_pool,
        ):
            accs = [
                acc_pool.tile([P, 3 * HALF], fp32, name="acc0", tag="acc0"),
                acc_pool.tile([P, 3 * (HALF - 1)], fp32, name="acc1", tag="acc1"),
                acc_pool.tile([P, 6], fp32, name="acc2", tag="acc2"),
            ]
            t0 = 0
            for cg in chunks:
                o_tile = io_pool.tile([P, 2 * D], fp32, name="o_tile")
                t_tile = io_pool.tile([P, 2 * D], fp32, name="t_tile")
                nc.sync.dma_start(
                    out=o_tile[:, 0 : cg * D].rearrange("p (t d) -> p t d", t=cg),
                    in_=o_all[:, t0 : t0 + cg],
                )
                nc.sync.dma_start(
                    out=t_tile[:, 0 : cg * D].rearrange("p (t d) -> p t d", t=cg),
                    in_=t_all[:, t0 : t0 + cg],
                )
                for gi in range(cg):
                    idx = t0 + gi
                    h = idx // HALF
                    col = 3 * (idx % HALF)
                    acc = accs[h]
                    osl = o_tile[:, gi * D : (gi + 1) * D]
                    tsl = t_tile[:, gi * D : (gi + 1) * D]
                    prod = scr_pool.tile([P, D], fp32, name="prod")
                    sq_o = scr_pool.tile([P, D], fp32, name="sq_o")
                    sq_t = scr_pool.tile([P, D], fp32, name="sq_t")
                    # dot: always DVE
                    nc.vector.scalar_tensor_tensor(
                        out=prod[:],
                        in0=osl,
                        scalar=1.0,
                        in1=tsl,
                        op0=mult,
                        op1=mult,
                        accum_out=acc[:, col : col + 1],
                    )
                    # o^2: always ACT
                    nc.scalar.activation(
                        sq_o[:], osl, Sq, accum_out=acc[:, col + 1 : col + 2]
                    )
                    # t^2: odd tiles on DVE, even on ACT (3 ACT / 3 DVE jobs
                    # per 2-tile chunk would overload DVE with dots; this
                    # gives DVE 15 dots + 7 squares, ACT 22 squares over
                    # tiles 0..14).
                    if idx % 2 == 1 or idx in (6, 12):
                        nc.vector.scalar_tensor_tensor(
                            out=sq_t[:],
                            in0=tsl,
                            scalar=1.0,
                            in1=tsl,
                            op0=mult,
                            op1=mult,
                            accum_out=acc[:, col + 2 : col + 3],
                        )
                    else:
                        nc.scalar.activation(
                            sq_t[:], tsl, Sq, accum_out=acc[:, col + 2 : col + 3]
                        )
                t0 += cg
            # tile 15 as two half-D loads; per half: dot on DVE, o^2 on ACT,
            # t^2 on DVE (h0) / ACT (h1) — both engines finish ~1.4 us after
            # the last bytes land
            hw = D // 2
            last = N_TILES - 1
            o_tile = io_pool.tile([P, 2 * D], fp32, name="o_tile")
            t_tile = io_pool.tile([P, 2 * D], fp32, name="t_tile")
            acc = accs[2]
            for j in range(2):
                dsl = slice(j * hw, (j + 1) * hw)
                nc.sync.dma_start(out=o_tile[:, j * hw : (j + 1) * hw], in_=o_all[:, last, dsl])
                nc.sync.dma_start(out=t_tile[:, j * hw : (j + 1) * hw], in_=t_all[:, last, dsl])
            for j in range(2):
                osl = o_tile[:, j * hw : (j + 1) * hw]
                tsl = t_tile[:, j * hw : (j + 1) * hw]
                prod = scr_pool.tile([P, hw], fp32, name="prod")
                sq_o = scr_pool.tile([P, hw], fp32, name="sq_o")
                sq_t = scr_pool.tile([P, hw], fp32, name="sq_t")
                nc.vector.scalar_tensor_tensor(
                    out=prod[:],
                    in0=osl,
                    scalar=1.0,
                    in1=tsl,
                    op0=mult,
                    op1=mult,
                    accum_out=acc[:, 3 * j : 3 * j + 1],
                )
                nc.scalar.activation(
                    sq_o[:], osl, Sq, accum_out=acc[:, 3 * j + 1 : 3 * j + 2]
                )
                if j == 0:
                    nc.vector.scalar_tensor_tensor(
                        out=sq_t[:],
                        in0=tsl,
                        scalar=1.0,
                        in1=tsl,
                        op0=mult,
                        op1=mult,
                        accum_out=acc[:, 3 * j + 2 : 3 * j + 3],
                    )
                else:
                    nc.scalar.activation(
                        sq_t[:], tsl, Sq, accum_out=acc[:, 3 * j + 2 : 3 * j + 3]
                    )
            # Emit the stats DMAs after every load so their compute-waits
            # stall the SP sequencer only once it has nothing left to issue.
            # stats0/1 still execute as soon as their tiles finish; stats2
            # (2 KiB) is the only write on the critical tail.
            nc.sync.dma_start(out=stats0[:, :], in_=accs[0][:])
            nc.sync.dma_start(out=stats1[:, :], in_=accs[1][:])
            nc.sync.dma_start(out=stats2[:, :], in_=accs[2][:])

    _trim_tail_barrier(nc)
    _hoist_first_loads(nc)
    _slim_exit_drain(nc)
    _legalize_waits(nc)
    _MAX_SEM["n"] = _compact_sems(nc) + 8  # headroom for walrus-internal sems
    return nc


def _get_nc():
    if "nc" not in _NC_CACHE:
        _install_walrus_flag_patch()
        _NC_CACHE["nc"] = _build_nc()
    return _NC_CACHE["nc"]


def _run_device(online_output, target_output, **spmd_kwargs):
    """Shard inputs, run the SPMD kernel, return per-core stats + raw result."""
    from concourse.bass_utils import run_bass_kernel_spmd

    nc = _get_nc()
    in_maps = []
    for c in range(N_CORES):
        sl = slice(c * N_LOC, (c + 1) * N_LOC)
        in_maps.append(
            {
                "online": np.ascontiguousarray(online_output[sl], dtype=np.float32),
                "target": np.ascontiguousarray(target_output[sl], dtype=np.float32),
            }
        )
    res = run_bass_kernel_spmd(nc, in_maps, list(range(N_CORES)), **spmd_kwargs)
    return res


def _finish_host(results):
    """Gather per-core stats and finish the cosine + mean in float64."""
    dots, n1s, n2s = [], [], []
    for i in range(N_CORES):
        st0 = np.asarray(results[i]["stats0"], dtype=np.float64)  # [P, 24]
        st1 = np.asarray(results[i]["stats1"], dtype=np.float64)  # [P, 21]
        st2 = np.asarray(results[i]["stats2"], dtype=np.float64)  # [P, 6]
        a0 = st0.reshape(P, HALF, 3)
        a1 = st1.reshape(P, HALF - 1, 3)
        a2 = (st2[:, 0:3] + st2[:, 3:6]).reshape(P, 1, 3)  # tile 15 halves
        # row_local = p*16 + t  ->  [P, 16, 3] flattens to row-major
        a = np.concatenate([a0, a1, a2], axis=1).reshape(-1, 3)
        dots.append(a[:, 0])
        n1s.append(a[:, 1])
        n2s.append(a[:, 2])
    dot = np.concatenate(dots)
    n1 = np.sqrt(np.concatenate(n1s))
    n2 = np.sqrt(np.concatenate(n2s))
    cos = dot / (np.maximum(n1, EPS) * np.maximum(n2, EPS))
    return np.array((2.0 - 2.0 * cos).mean() / TEMP, dtype=np.float32)


def kernel(online_output, target_output):
    res = _run_device(online_output, target_output)
    return _finish_host(res.results)
